# revision 16
# baseline (speedup 1.0000x reference)
"""Trainium2 Bass kernel for nn_ErdosLoss (graph loss function).

Math (reference reformulated, validated to ~1e-6 rel err):
  penalty:  log_score = scatter_add(log(1 - p + 1e-6), tgt)   over N nodes
            loss2 = mean(exp(log_score)) * 9600
  loss3:    p @ triu(H H^T, 1) @ p^T  ==  (||s||^2 - sum_e d_e p_e^2) / 2
            where s = scatter_add(p, tgt) + scatter_add(p * (1-m), src),
            m_e = (src_e == tgt_e)  (H rows are node *sets*: self-loops get a
            single 1), d_e = 2 - m_e.
  out = loss2 + 200 * loss3 / num_graphs,  num_graphs = max(batch) + 1.

Device strategy (8 NeuronCores, SPMD, two launches, no collectives):
  Measured on this stack: per-NEFF fixed overhead is ~12-13us (NRT EVSEM
  barriers + IRAM load + drains) and the 8 PJRT device launches skew by up
  to ~30us, so any cross-core barrier (AllReduce: ~13us + skew wait) costs
  ~45us of per-core exec time.  A barrier-free two-launch design wins:
  - Phase 1 (8 cores, edge-sharded 750/core, ~17us): scatter-add via
    one-hot matmul with node = 128*hi + lo decomposition (N padded to
    4096).  One-hots are built as a handful of *wide* DVE ops using
    stride-0 broadcast APs (per-instruction overhead dominates at this
    size) in f16 (exact for 0/1; value f16 quantization checked <=1e-3),
    then contracted on TensorE into PSUM [128lo, 64] (= log_score | s).
    iotas are generated on-device (gpsimd iota+cast); the Ln activation
    table is pre-warmed on memset data while the input DMA is in flight.
    Each core writes a [128, 65] f16 partial (log_score | s | dp2 rowsum).
  - Host gathers the 8 partials (pure data movement, c-innermost).
  - Phase 2 (1 core, ~16us): one strided 8-way reduce, exp/square row-sums
    (accum_out), ones-matmul partition reduce, num_graphs = max(batch)+1 on
    device (reduce_max over the tail of the sorted batch), final scalar
    arithmetic, out [1,1] f32.
  Total ~33us HW exec (sum of both launches); rel err ~4e-6 vs the fp32
  reference.  Engine-queue FIFO order is load-bearing: ops are emitted in
  critical-path order per engine.
"""

import numpy as np

import concourse.bacc as bacc
import concourse.mybir as mybir
import concourse.tile as tile
from concourse import bass_utils

F32 = mybir.dt.float32
F16 = mybir.dt.float16
ALU = mybir.AluOpType
ACT = mybir.ActivationFunctionType
AX = mybir.AxisListType

N_NODES = 4000
N_EDGES = 6000
N_CORES = 8
N_PAD = 4096          # 128 * 32
HI = 32               # node hi-digits
LO = 128              # node lo-digits
PENALTY_SCALE = 16 * 200 * 3   # 9600
PAD_NODES = N_PAD - N_NODES    # 96 padded nodes, each contributes exp(0)=1

EPC = N_EDGES // N_CORES       # 750 edges per core
TPC = (EPC + 127) // 128       # 6 edge tiles per core

def _build_phase1(T: int):
    """Per-core partial computation: out 'partial' [128, 65] f16."""
    nc = bacc.Bacc("TRN2", target_bir_lowering=False, debug=False, num_devices=1)

    # blob: [tlo,ulo,thi,uhi (4T) | tf,uf,p (3T)] -- iotas/constants are
    # generated on-device, so the only input DMA is the edge data itself
    blobd = nc.dram_tensor("blob", [128, 7 * T], F32, kind="ExternalInput").ap()
    partiald = nc.dram_tensor("partial", [128, 65], F16, kind="ExternalOutput").ap()

    with tile.TileContext(nc) as tc:
        with (
            tc.tile_pool(name="const", bufs=1) as cpool,
            tc.tile_pool(name="work", bufs=1) as wpool,
            tc.tile_pool(name="psum", bufs=1, space="PSUM") as ppool,
        ):
            # warm the Ln ACT table while the input DMA is in flight
            wz = cpool.tile([128, 1], F32, tag="wz")
            nc.vector.memset(wz[:], 0.5)
            wb = cpool.tile([128, 1], F32, tag="wb")
            nc.gpsimd.memset(wb[:], 0.0)
            bias1 = cpool.tile([128, 1], F32, tag="bias1")
            nc.gpsimd.memset(bias1[:], 1.0 + 1e-6)
            wo = cpool.tile([128, 1], F32, tag="wo")
            nc.scalar.activation(wo[:], wz[:], ACT.Ln, bias=wb[:])
            # iotas generated on-device (no DMA dependency)
            ioi = cpool.tile([128, LO], mybir.dt.int32, tag="ioi")
            nc.gpsimd.iota(ioi[:], pattern=[[1, LO]], base=0, channel_multiplier=0)
            io128t = cpool.tile([128, LO], F32, tag="io128t")
            nc.gpsimd.tensor_copy(io128t[:], ioi[:])
            io128 = io128t[:]
            io32 = io128t[:, 0:HI]

            bb = cpool.tile([128, 7 * T], F32, tag="bb")
            nc.sync.dma_start(bb[:], blobd)
            lo_pair = bb[:, 0:2 * T]
            hi_pair = bb[:, 2 * T:4 * T]
            tf = bb[:, 4 * T:5 * T]
            uf = bb[:, 5 * T:6 * T]
            pp = bb[:, 6 * T:7 * T]

            C = wpool.tile([128, 65], F16, tag="C")

            # ---- one-hots (f16, exact), few wide DVE ops via stride-0 APs
            H_all = wpool.tile([128, 2 * T * HI], F16, tag="H_all")
            nc.vector.tensor_tensor(
                H_all[:].rearrange("p (t h) -> p t h", h=HI),
                io32.rearrange("p (o h) -> p o h", o=1).to_broadcast((128, 2 * T, HI)),
                hi_pair.rearrange("p (t o) -> p t o", o=1).to_broadcast((128, 2 * T, HI)),
                op=ALU.is_equal,
            )
            A_all = wpool.tile([128, 2 * T * LO], F16, tag="A_all")
            nc.vector.tensor_tensor(
                A_all[:].rearrange("p (t l) -> p t l", l=LO),
                io128.rearrange("p (o l) -> p o l", o=1).to_broadcast((128, 2 * T, LO)),
                lo_pair.rearrange("p (t o) -> p t o", o=1).to_broadcast((128, 2 * T, LO)),
                op=ALU.is_equal,
            )
            # V = [logmsg | p] on the ACT engine (parallel to the DVE ops)
            V = wpool.tile([128, 2 * T], F32, tag="V")
            nc.scalar.activation(V[:, 0:T], pp, ACT.Ln, scale=-1.0, bias=bias1[:])
            nc.scalar.copy(V[:, T:2 * T], pp)

            # RS_all: per tile i the contiguous [rp_i(32) | rst_i(32)]
            RS_all = wpool.tile([128, T * 64], F16, tag="RS_all")
            nc.vector.tensor_tensor(
                RS_all[:].rearrange("p (t o h) -> p o t h", o=2, h=HI),
                H_all[:, 0:T * HI].rearrange("p (o t h) -> p o t h", o=1, h=HI)
                    .to_broadcast((128, 2, T, HI)),
                V[:].rearrange("p (o t) -> p o t", o=2)
                    .rearrange("p o (t h) -> p o t h", h=1)
                    .to_broadcast((128, 2, T, HI)),
                op=ALU.mult,
            )
            # small per-edge prep, after the MM1-critical ops in the DVE queue
            m = wpool.tile([128, T], F32, tag="m")
            nc.vector.tensor_tensor(m[:], tf, uf, op=ALU.is_equal)
            valu = wpool.tile([128, T], F32, tag="valu")   # p * (1 - m)
            nc.vector.scalar_tensor_tensor(
                valu[:], m[:], 0.5, pp, op0=ALU.is_lt, op1=ALU.mult
            )
            rsu_all = wpool.tile([128, T * HI], F16, tag="rsu_all")
            nc.vector.tensor_tensor(
                rsu_all[:].rearrange("p (t h) -> p t h", h=HI),
                H_all[:, T * HI:2 * T * HI].rearrange("p (t h) -> p t h", h=HI),
                valu[:].rearrange("p (t o) -> p t o", o=1).to_broadcast((128, T, HI)),
                op=ALU.mult,
            )
            # dp2 = p^2 (2 - m) = (valu + p) * p, row-summed (off critical path)
            tsum = wpool.tile([128, T], F32, tag="tsum")
            nc.vector.tensor_tensor(tsum[:], valu[:], pp, op=ALU.add)
            dp2scr = wpool.tile([128, T], F32, tag="dp2scr")
            dp2r = wpool.tile([128, 1], F32, tag="dp2r")
            nc.vector.scalar_tensor_tensor(
                dp2scr[:], tsum[:], 1.0, pp,
                op0=ALU.mult, op1=ALU.mult, accum_out=dp2r[:],
            )

            # ---- scatter-add matmuls: P12 = [log_score(32) | s(32)]
            P12 = ppool.tile([128, 64], F32, tag="P12")
            for i in range(T):
                nc.tensor.matmul(
                    P12[:, 0:64],
                    A_all[:, i * LO:(i + 1) * LO],
                    RS_all[:, i * 64:(i + 1) * 64],
                    start=(i == 0), stop=False, skip_group_check=True,
                )
            for i in range(T):
                nc.tensor.matmul(
                    P12[:, 32:64],
                    A_all[:, (T + i) * LO:(T + i + 1) * LO],
                    rsu_all[:, i * HI:(i + 1) * HI],
                    start=False, stop=(i == T - 1), skip_group_check=True,
                )

            nc.scalar.copy(C[:, 0:64], P12[:])
            nc.scalar.copy(C[:, 64:65], dp2r[:])
            nc.sync.dma_start(partiald, C[:])

    nc.compile()
    return nc


def _build_phase2():
    """Combine 8 partials -> final scalar. Runs on one core."""
    nc = bacc.Bacc("TRN2", target_bir_lowering=False, debug=False, num_devices=1)

    # parts: 8 x [128,65] partials, then cols 520:584 row 0 = batch[-64:]
    # (batch is sorted by construction so max(batch) = max of that tail;
    #  batch values < 32 are exact in f16)
    partsd = nc.dram_tensor("parts", [128, 584], F16, kind="ExternalInput").ap()
    outd = nc.dram_tensor("out", [1, 1], F32, kind="ExternalOutput").ap()

    with tile.TileContext(nc) as tc:
        with (
            tc.tile_pool(name="pool", bufs=1) as pool,
            tc.tile_pool(name="psum", bufs=1, space="PSUM") as ppool,
        ):
            wz = pool.tile([128, 1], F32, tag="wz")
            nc.vector.memset(wz[:], 0.5)
            wb = pool.tile([128, 1], F32, tag="wb")
            nc.gpsimd.memset(wb[:], 0.0)
            wo = pool.tile([128, 1], F32, tag="wo")
            nc.scalar.activation(wo[:], wz[:], ACT.Exp, bias=wb[:])

            ones_t = pool.tile([128, 1], F32, tag="ones_t")
            nc.gpsimd.memset(ones_t[:], 1.0)
            bzero = wb[:]

            pt = pool.tile([128, 584], F16, tag="pt")
            nc.sync.dma_start(pt[:], partsd)

            # 8-way partial sum in one reduce; host interleaves c innermost so
            # the 8 summands per output element are contiguous f16
            C2 = pool.tile([128, 65], F32, tag="C2")
            nc.vector.tensor_reduce(
                C2[:], pt[:, 0:520].rearrange("p (x c) -> p x c", c=8),
                axis=AX.X, op=ALU.add,
            )

            R = pool.tile([128, 3], F32, tag="R")
            scr1 = pool.tile([128, HI], F32, tag="scr1")
            nc.scalar.activation(scr1[:], C2[:, 0:32], ACT.Exp, bias=bzero,
                                 accum_out=R[:, 0:1])
            scr2 = pool.tile([128, HI], F32, tag="scr2")
            nc.vector.scalar_tensor_tensor(
                scr2[:], C2[:, 32:64], 1.0, C2[:, 32:64],
                op0=ALU.mult, op1=ALU.mult, accum_out=R[:, 1:2],
            )
            nc.vector.tensor_copy(R[:, 2:3], C2[:, 64:65])

            # num_graphs = max(batch)+1 via the sorted tail, partition 0 only
            ng = pool.tile([1, 1], F32, tag="ng")
            nc.vector.tensor_reduce(ng[:], pt[0:1, 520:584], axis=AX.X, op=ALU.max)
            ng1 = pool.tile([1, 1], F32, tag="ng1")
            nc.vector.tensor_scalar_add(ng1[:], ng[:], 1.0)
            rng = pool.tile([1, 1], F32, tag="rng")
            nc.vector.reciprocal(rng[:], ng1[:])

            F = ppool.tile([1, 3], F32, tag="F")
            nc.tensor.matmul(F[:], ones_t[:], R[:], start=True, stop=True)
            Fs = pool.tile([1, 2], F32, tag="Fs")
            nc.scalar.copy(Fs[:], F[:, 1:3])

            l2 = pool.tile([1, 1], F32, tag="l2")
            SC = PENALTY_SCALE / N_NODES
            nc.scalar.activation(l2[:], F[:, 0:1], ACT.Copy,
                                 bias=-float(PAD_NODES) * SC, scale=SC)
            d32 = pool.tile([1, 1], F32, tag="d32")
            nc.vector.tensor_tensor(d32[:], Fs[:, 0:1], Fs[:, 1:2], op=ALU.subtract)
            t2s = pool.tile([1, 1], F32, tag="t2s")
            nc.vector.scalar_tensor_tensor(
                t2s[:], d32[:], 100.0, rng[:], op0=ALU.mult, op1=ALU.mult
            )
            res = pool.tile([1, 1], F32, tag="res")
            nc.vector.tensor_tensor(res[:], l2[:], t2s[:], op=ALU.add)
            nc.sync.dma_start(outd, res[:])

    nc.compile()
    return nc


def _pack_core(tt, uu, p, T):
    """Pack one core's edge shard into the [128, 7*T] fp32 edata layout."""
    ne = tt.shape[0]
    npad = T * 128

    def pad(a, fill):
        out = np.full(npad, fill, np.float64)
        out[:ne] = a
        return out.reshape(T, 128).T.astype(np.float32)  # [128, T]

    t_lo = pad(tt % 128, 0.0)
    t_hi = pad(tt // 128, float(HI))     # sentinel hi -> matches nothing
    u_lo = pad(uu % 128, 0.0)
    u_hi = pad(uu // 128, float(HI))
    tf = pad(tt, 0.0)
    uf = pad(uu, 0.0)                    # pad: tf==uf -> m=1, but p=0
    pf = pad(p, 0.0)
    return np.concatenate([t_lo, u_lo, t_hi, u_hi, tf, uf, pf], axis=1)


_CACHE = {}


def _get(name, builder, *a):
    if name not in _CACHE:
        _CACHE[name] = builder(*a)
    return _CACHE[name]


def kernel(x, edge_index, edge_feature, batch, _trace=False):
    x = np.asarray(x)
    ei = np.asarray(edge_index).astype(np.int64)
    p = np.asarray(edge_feature).astype(np.float32)[:, 0]
    batch = np.asarray(batch).astype(np.int64)

    uu_all = ei[0].astype(np.float64)
    tt_all = ei[1].astype(np.float64)

    # ---- phase 1: per-core partials (no cross-core dependencies)
    nc1 = _get("p1", _build_phase1, TPC)
    in_maps = []
    for c in range(N_CORES):
        sl = slice(c * EPC, (c + 1) * EPC)
        in_maps.append({"blob": _pack_core(tt_all[sl], uu_all[sl], p[sl], TPC)})
    r1 = bass_utils.run_bass_kernel_spmd(
        nc1, in_maps, core_ids=list(range(N_CORES)), trace=_trace
    )

    # gather/unshard the per-core partials (pure data movement)
    parts = np.stack(
        [np.asarray(r1.results[c]["partial"]) for c in range(N_CORES)], axis=2
    ).reshape(128, 65 * N_CORES).astype(np.float16)   # [p, x*8+c], c innermost

    # ---- phase 2: combine on one core
    nc2 = _get("p2", _build_phase2)
    btail = np.zeros((128, 64), np.float16)
    btail[0, :] = batch[-64:].astype(np.float16)
    pts = np.concatenate([parts, btail], axis=1)
    r2 = bass_utils.run_bass_kernel_spmd(
        nc2, [{"parts": pts}], core_ids=[0], trace=_trace,
    )
    out = np.asarray(r2.results[0]["out"], dtype=np.float32).reshape(1, 1)
    if _trace:
        kernel.last_results = (r1, r2)
    return out


# revision 17
# speedup vs baseline: 1.0030x; 1.0030x over previous
"""Trainium2 Bass kernel for nn_ErdosLoss (graph loss function).

Math (reference reformulated, validated to ~1e-6 rel err):
  penalty:  log_score = scatter_add(log(1 - p + 1e-6), tgt)   over N nodes
            loss2 = mean(exp(log_score)) * 9600
  loss3:    p @ triu(H H^T, 1) @ p^T  ==  (||s||^2 - sum_e d_e p_e^2) / 2
            where s = scatter_add(p, tgt) + scatter_add(p * (1-m), src),
            m_e = (src_e == tgt_e)  (H rows are node *sets*: self-loops get a
            single 1), d_e = 2 - m_e.
  out = loss2 + 200 * loss3 / num_graphs,  num_graphs = max(batch) + 1.

Device strategy (8 NeuronCores, SPMD, two launches, no collectives):
  Measured on this stack: per-NEFF fixed overhead is ~12-13us (NRT EVSEM
  barriers + IRAM load + drains) and the 8 PJRT device launches skew by up
  to ~30us, so any cross-core barrier (AllReduce: ~13us + skew wait) costs
  ~45us of per-core exec time.  A barrier-free two-launch design wins:
  - Phase 1 (8 cores, edge-sharded 750/core, ~17us): scatter-add via
    one-hot matmul with node = 128*hi + lo decomposition (N padded to
    4096).  One-hots are built as a handful of *wide* DVE ops using
    stride-0 broadcast APs (per-instruction overhead dominates at this
    size) in f16 (exact for 0/1; value f16 quantization checked <=1e-3),
    then contracted on TensorE into PSUM [128lo, 64] (= log_score | s).
    iotas are generated on-device (gpsimd iota+cast); the Ln activation
    table is pre-warmed on memset data while the input DMA is in flight.
    Each core writes a [128, 65] f16 partial (log_score | s | dp2 rowsum).
  - Host gathers the 8 partials (pure data movement, c-innermost).
  - Phase 2 (1 core, ~16us): one strided 8-way reduce, exp/square row-sums
    (accum_out), ones-matmul partition reduce, num_graphs = max(batch)+1 on
    device (reduce_max over the tail of the sorted batch), final scalar
    arithmetic, out [1,1] f32.
  Total ~33us HW exec (sum of both launches); rel err ~4e-6 vs the fp32
  reference.  Engine-queue FIFO order is load-bearing: ops are emitted in
  critical-path order per engine.
"""

import numpy as np

import concourse.bacc as bacc
import concourse.mybir as mybir
import concourse.tile as tile
from concourse import bass_utils

F32 = mybir.dt.float32
F16 = mybir.dt.float16
ALU = mybir.AluOpType
ACT = mybir.ActivationFunctionType
AX = mybir.AxisListType

N_NODES = 4000
N_EDGES = 6000
N_CORES = 8
N_PAD = 4096          # 128 * 32
HI = 32               # node hi-digits
LO = 128              # node lo-digits
PENALTY_SCALE = 16 * 200 * 3   # 9600
PAD_NODES = N_PAD - N_NODES    # 96 padded nodes, each contributes exp(0)=1

EPC = N_EDGES // N_CORES       # 750 edges per core
TPC = (EPC + 127) // 128       # 6 edge tiles per core

def _build_phase1(T: int):
    """Per-core partial computation: out 'partial' [128, 65] f16."""
    nc = bacc.Bacc("TRN2", target_bir_lowering=False, debug=False, num_devices=1)

    # blob: [tlo,ulo,thi,uhi (4T) | tf,uf,p (3T)] -- iotas/constants are
    # generated on-device, so the only input DMA is the edge data itself
    blobd = nc.dram_tensor("blob", [128, 7 * T], F32, kind="ExternalInput").ap()
    partiald = nc.dram_tensor("partial", [128, 65], F16, kind="ExternalOutput").ap()

    with tile.TileContext(nc) as tc:
        with (
            tc.tile_pool(name="const", bufs=1) as cpool,
            tc.tile_pool(name="work", bufs=1) as wpool,
            tc.tile_pool(name="psum", bufs=1, space="PSUM") as ppool,
        ):
            # warm the Ln ACT table while the input DMA is in flight
            wz = cpool.tile([128, 1], F32, tag="wz")
            nc.vector.memset(wz[:], 0.5)
            wb = cpool.tile([128, 1], F32, tag="wb")
            nc.gpsimd.memset(wb[:], 0.0)
            bias1 = cpool.tile([128, 1], F32, tag="bias1")
            nc.gpsimd.memset(bias1[:], 1.0 + 1e-6)
            wo = cpool.tile([128, 1], F32, tag="wo")
            nc.scalar.activation(wo[:], wz[:], ACT.Ln, bias=wb[:])
            # iotas generated on-device (no DMA dependency)
            ioi = cpool.tile([128, LO], mybir.dt.int32, tag="ioi")
            nc.gpsimd.iota(ioi[:], pattern=[[1, LO]], base=0, channel_multiplier=0)
            io128t = cpool.tile([128, LO], F32, tag="io128t")
            nc.gpsimd.tensor_copy(io128t[:], ioi[:])
            io128 = io128t[:]
            io32 = io128t[:, 0:HI]

            bb = cpool.tile([128, 7 * T], F32, tag="bb")
            nc.sync.dma_start(bb[:], blobd)
            lo_pair = bb[:, 0:2 * T]
            hi_pair = bb[:, 2 * T:4 * T]
            tf = bb[:, 4 * T:5 * T]
            uf = bb[:, 5 * T:6 * T]
            pp = bb[:, 6 * T:7 * T]

            C = wpool.tile([128, 65], F16, tag="C")

            # ---- one-hots (f16, exact), few wide DVE ops via stride-0 APs
            H_all = wpool.tile([128, 2 * T * HI], F16, tag="H_all")
            nc.vector.tensor_tensor(
                H_all[:].rearrange("p (t h) -> p t h", h=HI),
                io32.rearrange("p (o h) -> p o h", o=1).to_broadcast((128, 2 * T, HI)),
                hi_pair.rearrange("p (t o) -> p t o", o=1).to_broadcast((128, 2 * T, HI)),
                op=ALU.is_equal,
            )
            # lo one-hot, target half first so the MM1 group can start early
            A_all = wpool.tile([128, 2 * T * LO], F16, tag="A_all")
            nc.vector.tensor_tensor(
                A_all[:, 0:T * LO].rearrange("p (t l) -> p t l", l=LO),
                io128.rearrange("p (o l) -> p o l", o=1).to_broadcast((128, T, LO)),
                lo_pair[:, 0:T].rearrange("p (t o) -> p t o", o=1)
                    .to_broadcast((128, T, LO)),
                op=ALU.is_equal,
            )
            # V = [logmsg | p] on the ACT engine (parallel to the DVE ops)
            V = wpool.tile([128, 2 * T], F32, tag="V")
            nc.scalar.activation(V[:, 0:T], pp, ACT.Ln, scale=-1.0, bias=bias1[:])
            nc.scalar.copy(V[:, T:2 * T], pp)

            # RS_all: per tile i the contiguous [rp_i(32) | rst_i(32)]
            RS_all = wpool.tile([128, T * 64], F16, tag="RS_all")
            nc.vector.tensor_tensor(
                RS_all[:].rearrange("p (t o h) -> p o t h", o=2, h=HI),
                H_all[:, 0:T * HI].rearrange("p (o t h) -> p o t h", o=1, h=HI)
                    .to_broadcast((128, 2, T, HI)),
                V[:].rearrange("p (o t) -> p o t", o=2)
                    .rearrange("p o (t h) -> p o t h", h=1)
                    .to_broadcast((128, 2, T, HI)),
                op=ALU.mult,
            )
            # source half of the lo one-hot + small per-edge prep
            nc.vector.tensor_tensor(
                A_all[:, T * LO:2 * T * LO].rearrange("p (t l) -> p t l", l=LO),
                io128.rearrange("p (o l) -> p o l", o=1).to_broadcast((128, T, LO)),
                lo_pair[:, T:2 * T].rearrange("p (t o) -> p t o", o=1)
                    .to_broadcast((128, T, LO)),
                op=ALU.is_equal,
            )
            m = wpool.tile([128, T], F32, tag="m")
            nc.vector.tensor_tensor(m[:], tf, uf, op=ALU.is_equal)
            valu = wpool.tile([128, T], F32, tag="valu")   # p * (1 - m)
            nc.vector.scalar_tensor_tensor(
                valu[:], m[:], 0.5, pp, op0=ALU.is_lt, op1=ALU.mult
            )
            rsu_all = wpool.tile([128, T * HI], F16, tag="rsu_all")
            nc.vector.tensor_tensor(
                rsu_all[:].rearrange("p (t h) -> p t h", h=HI),
                H_all[:, T * HI:2 * T * HI].rearrange("p (t h) -> p t h", h=HI),
                valu[:].rearrange("p (t o) -> p t o", o=1).to_broadcast((128, T, HI)),
                op=ALU.mult,
            )
            # dp2 = p^2 (2 - m) = (valu + p) * p, row-summed (off critical path)
            tsum = wpool.tile([128, T], F32, tag="tsum")
            nc.vector.tensor_tensor(tsum[:], valu[:], pp, op=ALU.add)
            dp2scr = wpool.tile([128, T], F32, tag="dp2scr")
            dp2r = wpool.tile([128, 1], F32, tag="dp2r")
            nc.vector.scalar_tensor_tensor(
                dp2scr[:], tsum[:], 1.0, pp,
                op0=ALU.mult, op1=ALU.mult, accum_out=dp2r[:],
            )

            # ---- scatter-add matmuls: P12 = [log_score(32) | s(32)]
            P12 = ppool.tile([128, 64], F32, tag="P12")
            for i in range(T):
                nc.tensor.matmul(
                    P12[:, 0:64],
                    A_all[:, i * LO:(i + 1) * LO],
                    RS_all[:, i * 64:(i + 1) * 64],
                    start=(i == 0), stop=False, skip_group_check=True,
                )
            for i in range(T):
                nc.tensor.matmul(
                    P12[:, 32:64],
                    A_all[:, (T + i) * LO:(T + i + 1) * LO],
                    rsu_all[:, i * HI:(i + 1) * HI],
                    start=False, stop=(i == T - 1), skip_group_check=True,
                )

            nc.scalar.copy(C[:, 0:64], P12[:])
            nc.scalar.copy(C[:, 64:65], dp2r[:])
            nc.sync.dma_start(partiald, C[:])

    nc.compile()
    return nc


def _build_phase2():
    """Combine 8 partials -> final scalar. Runs on one core."""
    nc = bacc.Bacc("TRN2", target_bir_lowering=False, debug=False, num_devices=1)

    # parts: 8 x [128,65] partials, then cols 520:584 row 0 = batch[-64:]
    # (batch is sorted by construction so max(batch) = max of that tail;
    #  batch values < 32 are exact in f16)
    partsd = nc.dram_tensor("parts", [128, 584], F16, kind="ExternalInput").ap()
    outd = nc.dram_tensor("out", [1, 1], F32, kind="ExternalOutput").ap()

    with tile.TileContext(nc) as tc:
        with (
            tc.tile_pool(name="pool", bufs=1) as pool,
            tc.tile_pool(name="psum", bufs=1, space="PSUM") as ppool,
        ):
            wz = pool.tile([128, 1], F32, tag="wz")
            nc.vector.memset(wz[:], 0.5)
            wb = pool.tile([128, 1], F32, tag="wb")
            nc.gpsimd.memset(wb[:], 0.0)
            wo = pool.tile([128, 1], F32, tag="wo")
            nc.scalar.activation(wo[:], wz[:], ACT.Exp, bias=wb[:])

            ones_t = pool.tile([128, 1], F32, tag="ones_t")
            nc.gpsimd.memset(ones_t[:], 1.0)
            bzero = wb[:]

            pt = pool.tile([128, 584], F16, tag="pt")
            nc.sync.dma_start(pt[:], partsd)

            # 8-way partial sum in one reduce; host interleaves c innermost so
            # the 8 summands per output element are contiguous f16
            C2 = pool.tile([128, 65], F32, tag="C2")
            nc.vector.tensor_reduce(
                C2[:], pt[:, 0:520].rearrange("p (x c) -> p x c", c=8),
                axis=AX.X, op=ALU.add,
            )

            R = pool.tile([128, 3], F32, tag="R")
            scr1 = pool.tile([128, HI], F32, tag="scr1")
            nc.scalar.activation(scr1[:], C2[:, 0:32], ACT.Exp, bias=bzero,
                                 accum_out=R[:, 0:1])
            scr2 = pool.tile([128, HI], F32, tag="scr2")
            nc.vector.scalar_tensor_tensor(
                scr2[:], C2[:, 32:64], 1.0, C2[:, 32:64],
                op0=ALU.mult, op1=ALU.mult, accum_out=R[:, 1:2],
            )
            nc.vector.tensor_copy(R[:, 2:3], C2[:, 64:65])

            # num_graphs = max(batch)+1 via the sorted tail, partition 0 only
            ng = pool.tile([1, 1], F32, tag="ng")
            nc.vector.tensor_reduce(ng[:], pt[0:1, 520:584], axis=AX.X, op=ALU.max)
            ng1 = pool.tile([1, 1], F32, tag="ng1")
            nc.vector.tensor_scalar_add(ng1[:], ng[:], 1.0)
            rng = pool.tile([1, 1], F32, tag="rng")
            nc.vector.reciprocal(rng[:], ng1[:])

            F = ppool.tile([1, 3], F32, tag="F")
            nc.tensor.matmul(F[:], ones_t[:], R[:], start=True, stop=True)
            Fs = pool.tile([1, 2], F32, tag="Fs")
            nc.scalar.copy(Fs[:], F[:, 1:3])

            l2 = pool.tile([1, 1], F32, tag="l2")
            SC = PENALTY_SCALE / N_NODES
            nc.scalar.activation(l2[:], F[:, 0:1], ACT.Copy,
                                 bias=-float(PAD_NODES) * SC, scale=SC)
            d32 = pool.tile([1, 1], F32, tag="d32")
            nc.vector.tensor_tensor(d32[:], Fs[:, 0:1], Fs[:, 1:2], op=ALU.subtract)
            t2s = pool.tile([1, 1], F32, tag="t2s")
            nc.vector.scalar_tensor_tensor(
                t2s[:], d32[:], 100.0, rng[:], op0=ALU.mult, op1=ALU.mult
            )
            res = pool.tile([1, 1], F32, tag="res")
            nc.vector.tensor_tensor(res[:], l2[:], t2s[:], op=ALU.add)
            nc.sync.dma_start(outd, res[:])

    nc.compile()
    return nc


def _pack_core(tt, uu, p, T):
    """Pack one core's edge shard into the [128, 7*T] fp32 edata layout."""
    ne = tt.shape[0]
    npad = T * 128

    def pad(a, fill):
        out = np.full(npad, fill, np.float64)
        out[:ne] = a
        return out.reshape(T, 128).T.astype(np.float32)  # [128, T]

    t_lo = pad(tt % 128, 0.0)
    t_hi = pad(tt // 128, float(HI))     # sentinel hi -> matches nothing
    u_lo = pad(uu % 128, 0.0)
    u_hi = pad(uu // 128, float(HI))
    tf = pad(tt, 0.0)
    uf = pad(uu, 0.0)                    # pad: tf==uf -> m=1, but p=0
    pf = pad(p, 0.0)
    return np.concatenate([t_lo, u_lo, t_hi, u_hi, tf, uf, pf], axis=1)


_CACHE = {}


def _get(name, builder, *a):
    if name not in _CACHE:
        _CACHE[name] = builder(*a)
    return _CACHE[name]


def kernel(x, edge_index, edge_feature, batch, _trace=False):
    x = np.asarray(x)
    ei = np.asarray(edge_index).astype(np.int64)
    p = np.asarray(edge_feature).astype(np.float32)[:, 0]
    batch = np.asarray(batch).astype(np.int64)

    uu_all = ei[0].astype(np.float64)
    tt_all = ei[1].astype(np.float64)

    # ---- phase 1: per-core partials (no cross-core dependencies)
    nc1 = _get("p1", _build_phase1, TPC)
    in_maps = []
    for c in range(N_CORES):
        sl = slice(c * EPC, (c + 1) * EPC)
        in_maps.append({"blob": _pack_core(tt_all[sl], uu_all[sl], p[sl], TPC)})
    r1 = bass_utils.run_bass_kernel_spmd(
        nc1, in_maps, core_ids=list(range(N_CORES)), trace=_trace
    )

    # gather/unshard the per-core partials (pure data movement)
    parts = np.stack(
        [np.asarray(r1.results[c]["partial"]) for c in range(N_CORES)], axis=2
    ).reshape(128, 65 * N_CORES).astype(np.float16)   # [p, x*8+c], c innermost

    # ---- phase 2: combine on one core
    nc2 = _get("p2", _build_phase2)
    btail = np.zeros((128, 64), np.float16)
    btail[0, :] = batch[-64:].astype(np.float16)
    pts = np.concatenate([parts, btail], axis=1)
    r2 = bass_utils.run_bass_kernel_spmd(
        nc2, [{"parts": pts}], core_ids=[0], trace=_trace,
    )
    out = np.asarray(r2.results[0]["out"], dtype=np.float32).reshape(1, 1)
    if _trace:
        kernel.last_results = (r1, r2)
    return out


# revision 18
# speedup vs baseline: 1.0273x; 1.0242x over previous
"""Trainium2 Bass kernel for nn_ErdosLoss (graph loss function).

Math (reference reformulated, validated to ~1e-6 rel err):
  penalty:  log_score = scatter_add(log(1 - p + 1e-6), tgt)   over N nodes
            loss2 = mean(exp(log_score)) * 9600
  loss3:    p @ triu(H H^T, 1) @ p^T  ==  (||s||^2 - sum_e d_e p_e^2) / 2
            where s = scatter_add(p, tgt) + scatter_add(p * (1-m), src),
            m_e = (src_e == tgt_e)  (H rows are node *sets*: self-loops get a
            single 1), d_e = 2 - m_e.
  out = loss2 + 200 * loss3 / num_graphs,  num_graphs = max(batch) + 1.

Device strategy (8 NeuronCores, SPMD, two launches, no collectives):
  Measured on this stack: per-NEFF fixed overhead is ~12-13us (NRT EVSEM
  barriers + IRAM load + drains) and the 8 PJRT device launches skew by up
  to ~30us, so any cross-core barrier (AllReduce: ~13us + skew wait) costs
  ~45us of per-core exec time.  A barrier-free two-launch design wins:
  - Phase 1 (8 cores, edge-sharded 750/core, ~17us): scatter-add via
    one-hot matmul with node = 128*hi + lo decomposition (N padded to
    4096).  One-hots are built as a handful of *wide* DVE ops using
    stride-0 broadcast APs (per-instruction overhead dominates at this
    size) in f16 (exact for 0/1; value f16 quantization checked <=1e-3),
    then contracted on TensorE into PSUM [128lo, 64] (= log_score | s).
    iotas are generated on-device (gpsimd iota+cast); the Ln activation
    table is pre-warmed on memset data while the input DMA is in flight.
    Each core writes a [128, 65] f16 partial (log_score | s | dp2 rowsum).
  - Host gathers the 8 partials (pure data movement, c-innermost).
  - Phase 2 (1 core, ~16us): one strided 8-way reduce, exp/square row-sums
    (accum_out), ones-matmul partition reduce, num_graphs = max(batch)+1 on
    device (reduce_max over the tail of the sorted batch), final scalar
    arithmetic, out [1,1] f32.
  Total ~33us HW exec (sum of both launches); rel err ~4e-6 vs the fp32
  reference.  Engine-queue FIFO order is load-bearing: ops are emitted in
  critical-path order per engine.
"""

import numpy as np

import concourse.bacc as bacc
import concourse.mybir as mybir
import concourse.tile as tile
from concourse import bass_utils

F32 = mybir.dt.float32
F16 = mybir.dt.float16
ALU = mybir.AluOpType
ACT = mybir.ActivationFunctionType
AX = mybir.AxisListType

N_NODES = 4000
N_EDGES = 6000
N_CORES = 8
N_PAD = 4096          # 128 * 32
HI = 32               # node hi-digits
LO = 128              # node lo-digits
PENALTY_SCALE = 16 * 200 * 3   # 9600
PAD_NODES = N_PAD - N_NODES    # 96 padded nodes, each contributes exp(0)=1

EPC = N_EDGES // N_CORES       # 750 edges per core
TPC = (EPC + 127) // 128       # 6 edge tiles per core

def _build_phase1(T: int):
    """Per-core partial computation: out 'partial' [128, 65] f16."""
    nc = bacc.Bacc("TRN2", target_bir_lowering=False, debug=False, num_devices=1)

    # blob: [tlo,ulo,thi,uhi (4T) | tf,uf,p (3T)] -- iotas/constants are
    # generated on-device, so the only input DMA is the edge data itself
    blobd = nc.dram_tensor("blob", [128, 7 * T], F32, kind="ExternalInput").ap()
    partiald = nc.dram_tensor("partial", [128, 65], F16, kind="ExternalOutput").ap()

    with tile.TileContext(nc) as tc:
        with (
            tc.tile_pool(name="const", bufs=1) as cpool,
            tc.tile_pool(name="work", bufs=1) as wpool,
            tc.tile_pool(name="psum", bufs=1, space="PSUM") as ppool,
        ):
            # warm the Ln ACT table while the input DMA is in flight
            wz = cpool.tile([128, 1], F32, tag="wz")
            nc.vector.memset(wz[:], 0.5)
            wb = cpool.tile([128, 1], F32, tag="wb")
            nc.gpsimd.memset(wb[:], 0.0)
            bias1 = cpool.tile([128, 1], F32, tag="bias1")
            nc.gpsimd.memset(bias1[:], 1.0 + 1e-6)
            wo = cpool.tile([128, 1], F32, tag="wo")
            nc.scalar.activation(wo[:], wz[:], ACT.Ln, bias=wb[:])
            # iotas generated on-device (no DMA dependency)
            ioi = cpool.tile([128, LO], mybir.dt.int32, tag="ioi")
            nc.gpsimd.iota(ioi[:], pattern=[[1, LO]], base=0, channel_multiplier=0)
            io128t = cpool.tile([128, LO], F32, tag="io128t")
            nc.gpsimd.tensor_copy(io128t[:], ioi[:])
            io128 = io128t[:]
            io32 = io128t[:, 0:HI]

            bb = cpool.tile([128, 7 * T], F32, tag="bb")
            nc.sync.dma_start(bb[:], blobd)
            lo_pair = bb[:, 0:2 * T]
            hi_pair = bb[:, 2 * T:4 * T]
            tf = bb[:, 4 * T:5 * T]
            uf = bb[:, 5 * T:6 * T]
            pp = bb[:, 6 * T:7 * T]

            C = wpool.tile([128, 65], F16, tag="C")

            # ---- one-hots (f16, exact), few wide DVE ops via stride-0 APs
            H_all = wpool.tile([128, 2 * T * HI], F16, tag="H_all")
            nc.vector.tensor_tensor(
                H_all[:].rearrange("p (t h) -> p t h", h=HI),
                io32.rearrange("p (o h) -> p o h", o=1).to_broadcast((128, 2 * T, HI)),
                hi_pair.rearrange("p (t o) -> p t o", o=1).to_broadcast((128, 2 * T, HI)),
                op=ALU.is_equal,
            )
            # lo one-hot, target half first so the MM1 group can start early
            A_all = wpool.tile([128, 2 * T * LO], F16, tag="A_all")
            nc.vector.tensor_tensor(
                A_all[:, 0:T * LO].rearrange("p (t l) -> p t l", l=LO),
                io128.rearrange("p (o l) -> p o l", o=1).to_broadcast((128, T, LO)),
                lo_pair[:, 0:T].rearrange("p (t o) -> p t o", o=1)
                    .to_broadcast((128, T, LO)),
                op=ALU.is_equal,
            )
            # V = [logmsg | p] on the ACT engine (parallel to the DVE ops)
            V = wpool.tile([128, 2 * T], F32, tag="V")
            nc.scalar.activation(V[:, 0:T], pp, ACT.Ln, scale=-1.0, bias=bias1[:])
            nc.scalar.copy(V[:, T:2 * T], pp)

            # RS_all: per tile i the contiguous [rp_i(32) | rst_i(32)]
            RS_all = wpool.tile([128, T * 64], F16, tag="RS_all")
            nc.vector.tensor_tensor(
                RS_all[:].rearrange("p (t o h) -> p o t h", o=2, h=HI),
                H_all[:, 0:T * HI].rearrange("p (o t h) -> p o t h", o=1, h=HI)
                    .to_broadcast((128, 2, T, HI)),
                V[:].rearrange("p (o t) -> p o t", o=2)
                    .rearrange("p o (t h) -> p o t h", h=1)
                    .to_broadcast((128, 2, T, HI)),
                op=ALU.mult,
            )
            # source half of the lo one-hot + small per-edge prep
            nc.vector.tensor_tensor(
                A_all[:, T * LO:2 * T * LO].rearrange("p (t l) -> p t l", l=LO),
                io128.rearrange("p (o l) -> p o l", o=1).to_broadcast((128, T, LO)),
                lo_pair[:, T:2 * T].rearrange("p (t o) -> p t o", o=1)
                    .to_broadcast((128, T, LO)),
                op=ALU.is_equal,
            )
            m = wpool.tile([128, T], F32, tag="m")
            nc.vector.tensor_tensor(m[:], tf, uf, op=ALU.is_equal)
            valu = wpool.tile([128, T], F32, tag="valu")   # p * (1 - m)
            nc.vector.scalar_tensor_tensor(
                valu[:], m[:], 0.5, pp, op0=ALU.is_lt, op1=ALU.mult
            )
            rsu_all = wpool.tile([128, T * HI], F16, tag="rsu_all")
            nc.vector.tensor_tensor(
                rsu_all[:].rearrange("p (t h) -> p t h", h=HI),
                H_all[:, T * HI:2 * T * HI].rearrange("p (t h) -> p t h", h=HI),
                valu[:].rearrange("p (t o) -> p t o", o=1).to_broadcast((128, T, HI)),
                op=ALU.mult,
            )
            # dp2 = p^2 (2 - m) = (valu + p) * p, row-summed (off critical path)
            tsum = wpool.tile([128, T], F32, tag="tsum")
            nc.vector.tensor_tensor(tsum[:], valu[:], pp, op=ALU.add)
            dp2scr = wpool.tile([128, T], F32, tag="dp2scr")
            dp2r = wpool.tile([128, 1], F32, tag="dp2r")
            nc.vector.scalar_tensor_tensor(
                dp2scr[:], tsum[:], 1.0, pp,
                op0=ALU.mult, op1=ALU.mult, accum_out=dp2r[:],
            )

            # ---- scatter-add matmuls: P12 = [log_score(32) | s(32)]
            P12 = ppool.tile([128, 64], F32, tag="P12")
            for i in range(T):
                nc.tensor.matmul(
                    P12[:, 0:64],
                    A_all[:, i * LO:(i + 1) * LO],
                    RS_all[:, i * 64:(i + 1) * 64],
                    start=(i == 0), stop=False, skip_group_check=True,
                )
            for i in range(T):
                nc.tensor.matmul(
                    P12[:, 32:64],
                    A_all[:, (T + i) * LO:(T + i + 1) * LO],
                    rsu_all[:, i * HI:(i + 1) * HI],
                    start=False, stop=(i == T - 1), skip_group_check=True,
                )

            nc.scalar.copy(C[:, 0:64], P12[:])
            nc.gpsimd.tensor_copy(C[:, 64:65], dp2r[:])
            nc.sync.dma_start(partiald, C[:])

    nc.compile()
    return nc


def _build_phase2():
    """Combine 8 partials -> final scalar. Runs on one core."""
    nc = bacc.Bacc("TRN2", target_bir_lowering=False, debug=False, num_devices=1)

    # partials, c innermost: partsa = x 0:32 (log_score), partsb = x 32:65
    # (s | dp2) then 64 cols whose row 0 holds batch[-64:] (batch is sorted
    # by construction, so max(batch) = max of that tail; values < 32 are
    # exact in f16)
    partsad = nc.dram_tensor("partsa", [128, 256], F16, kind="ExternalInput").ap()
    partsbd = nc.dram_tensor("partsb", [128, 328], F16, kind="ExternalInput").ap()
    outd = nc.dram_tensor("out", [1, 1], F32, kind="ExternalOutput").ap()

    with tile.TileContext(nc) as tc:
        with (
            tc.tile_pool(name="pool", bufs=1) as pool,
            tc.tile_pool(name="psum", bufs=1, space="PSUM") as ppool,
        ):
            wz = pool.tile([128, 1], F32, tag="wz")
            nc.vector.memset(wz[:], 0.5)
            wb = pool.tile([128, 1], F32, tag="wb")
            nc.gpsimd.memset(wb[:], 0.0)
            wo = pool.tile([128, 1], F32, tag="wo")
            nc.scalar.activation(wo[:], wz[:], ACT.Exp, bias=wb[:])

            ones_t = pool.tile([128, 1], F32, tag="ones_t")
            nc.gpsimd.memset(ones_t[:], 1.0)
            bzero = wb[:]

            # two input DMAs on independent queues (sync + gpsimd)
            pta = pool.tile([128, 256], F16, tag="pta")
            nc.sync.dma_start(pta[:], partsad)
            ptb = pool.tile([128, 328], F16, tag="ptb")
            nc.gpsimd.dma_start(ptb[:], partsbd)

            # 8-way partial sums; the log_score half unblocks EXP first
            C2a = pool.tile([128, 32], F32, tag="C2a")
            nc.vector.tensor_reduce(
                C2a[:], pta[:].rearrange("p (x c) -> p x c", c=8),
                axis=AX.X, op=ALU.add,
            )
            C2b = pool.tile([128, 33], F32, tag="C2b")
            nc.vector.tensor_reduce(
                C2b[:], ptb[:, 0:264].rearrange("p (x c) -> p x c", c=8),
                axis=AX.X, op=ALU.add,
            )

            R = pool.tile([128, 3], F32, tag="R")
            scr1 = pool.tile([128, HI], F32, tag="scr1")
            nc.scalar.activation(scr1[:], C2a[:], ACT.Exp, bias=bzero,
                                 accum_out=R[:, 0:1])
            scr2 = pool.tile([128, HI], F32, tag="scr2")
            nc.vector.scalar_tensor_tensor(
                scr2[:], C2b[:, 0:32], 1.0, C2b[:, 0:32],
                op0=ALU.mult, op1=ALU.mult, accum_out=R[:, 1:2],
            )
            nc.vector.tensor_copy(R[:, 2:3], C2b[:, 32:33])

            # num_graphs: rng = 100 / (max(batch) + 1), off the critical path
            ng = pool.tile([1, 1], F32, tag="ng")
            nc.vector.tensor_reduce(ng[:], ptb[0:1, 264:328], axis=AX.X, op=ALU.max)
            ng1 = pool.tile([1, 1], F32, tag="ng1")
            nc.vector.tensor_scalar(ng1[:], ng[:], 1.0, 0.01, op0=ALU.add, op1=ALU.mult)
            rng = pool.tile([1, 1], F32, tag="rng")
            nc.vector.reciprocal(rng[:], ng1[:])

            F = ppool.tile([1, 3], F32, tag="F")
            nc.tensor.matmul(F[:], ones_t[:], R[:], start=True, stop=True)
            Fs = pool.tile([1, 2], F32, tag="Fs")
            nc.scalar.copy(Fs[:], F[:, 1:3])

            l2 = pool.tile([1, 1], F32, tag="l2")
            SC = PENALTY_SCALE / N_NODES
            nc.scalar.activation(l2[:], F[:, 0:1], ACT.Copy,
                                 bias=-float(PAD_NODES) * SC, scale=SC)
            d32 = pool.tile([1, 1], F32, tag="d32")
            nc.vector.tensor_tensor(d32[:], Fs[:, 0:1], Fs[:, 1:2], op=ALU.subtract)
            # res = d32 * (100/ng) + l2 in one fused op (scalar is an AP)
            res = pool.tile([1, 1], F32, tag="res")
            nc.vector.scalar_tensor_tensor(
                res[:], d32[:], rng[:], l2[:], op0=ALU.mult, op1=ALU.add
            )
            nc.sync.dma_start(outd, res[:])

    nc.compile()
    return nc


def _pack_core(tt, uu, p, T):
    """Pack one core's edge shard into the [128, 7*T] fp32 edata layout."""
    ne = tt.shape[0]
    npad = T * 128

    def pad(a, fill):
        out = np.full(npad, fill, np.float64)
        out[:ne] = a
        return out.reshape(T, 128).T.astype(np.float32)  # [128, T]

    t_lo = pad(tt % 128, 0.0)
    t_hi = pad(tt // 128, float(HI))     # sentinel hi -> matches nothing
    u_lo = pad(uu % 128, 0.0)
    u_hi = pad(uu // 128, float(HI))
    tf = pad(tt, 0.0)
    uf = pad(uu, 0.0)                    # pad: tf==uf -> m=1, but p=0
    pf = pad(p, 0.0)
    return np.concatenate([t_lo, u_lo, t_hi, u_hi, tf, uf, pf], axis=1)


_CACHE = {}


def _get(name, builder, *a):
    if name not in _CACHE:
        _CACHE[name] = builder(*a)
    return _CACHE[name]


def kernel(x, edge_index, edge_feature, batch, _trace=False):
    x = np.asarray(x)
    ei = np.asarray(edge_index).astype(np.int64)
    p = np.asarray(edge_feature).astype(np.float32)[:, 0]
    batch = np.asarray(batch).astype(np.int64)

    uu_all = ei[0].astype(np.float64)
    tt_all = ei[1].astype(np.float64)

    # ---- phase 1: per-core partials (no cross-core dependencies)
    nc1 = _get("p1", _build_phase1, TPC)
    in_maps = []
    for c in range(N_CORES):
        sl = slice(c * EPC, (c + 1) * EPC)
        in_maps.append({"blob": _pack_core(tt_all[sl], uu_all[sl], p[sl], TPC)})
    r1 = bass_utils.run_bass_kernel_spmd(
        nc1, in_maps, core_ids=list(range(N_CORES)), trace=_trace
    )

    # gather/unshard the per-core partials (pure data movement)
    parts = np.stack(
        [np.asarray(r1.results[c]["partial"]) for c in range(N_CORES)], axis=2
    ).astype(np.float16)                               # [p, x, c], c innermost

    # ---- phase 2: combine on one core
    nc2 = _get("p2", _build_phase2)
    btail = np.zeros((128, 64), np.float16)
    btail[0, :] = batch[-64:].astype(np.float16)
    partsa = parts[:, 0:32, :].reshape(128, 256)
    partsb = np.concatenate([parts[:, 32:65, :].reshape(128, 264), btail], axis=1)
    r2 = bass_utils.run_bass_kernel_spmd(
        nc2, [{"partsa": partsa, "partsb": partsb}], core_ids=[0], trace=_trace,
    )
    out = np.asarray(r2.results[0]["out"], dtype=np.float32).reshape(1, 1)
    if _trace:
        kernel.last_results = (r1, r2)
    return out


# revision 19
# speedup vs baseline: 1.0275x; 1.0003x over previous
"""Trainium2 Bass kernel for nn_ErdosLoss (graph loss function).

Math (reference reformulated, validated to ~1e-6 rel err):
  penalty:  log_score = scatter_add(log(1 - p + 1e-6), tgt)   over N nodes
            loss2 = mean(exp(log_score)) * 9600
  loss3:    p @ triu(H H^T, 1) @ p^T  ==  (||s||^2 - sum_e d_e p_e^2) / 2
            where s = scatter_add(p, tgt) + scatter_add(p * (1-m), src),
            m_e = (src_e == tgt_e)  (H rows are node *sets*: self-loops get a
            single 1), d_e = 2 - m_e.
  out = loss2 + 200 * loss3 / num_graphs,  num_graphs = max(batch) + 1.

Device strategy (8 NeuronCores, SPMD, two launches, no collectives):
  Measured on this stack: per-NEFF fixed overhead is ~12-13us (NRT EVSEM
  barriers + IRAM load + drains) and the 8 PJRT device launches skew by up
  to ~30us, so any cross-core barrier (AllReduce: ~13us + skew wait) costs
  ~45us of per-core exec time.  A barrier-free two-launch design wins:
  - Phase 1 (8 cores, edge-sharded 750/core, ~17us): scatter-add via
    one-hot matmul with node = 128*hi + lo decomposition (N padded to
    4096).  One-hots are built as a handful of *wide* DVE ops using
    stride-0 broadcast APs (per-instruction overhead dominates at this
    size) in f16 (exact for 0/1; value f16 quantization checked <=1e-3),
    then contracted on TensorE into PSUM [128lo, 64] (= log_score | s).
    iotas are generated on-device (gpsimd iota+cast); the Ln activation
    table is pre-warmed on memset data while the input DMA is in flight.
    Each core writes a [128, 65] f16 partial (log_score | s | dp2 rowsum).
  - Host gathers the 8 partials (pure data movement, c-innermost, split
    into the log_score half and the s|dp2|batch-tail half).
  - Phase 2 (1 core, ~15.4us): two input DMAs on independent queues, two
    strided 8-way reduces (the exp-feeding half unblocks first), exp/square
    row-sums (accum_out), ones-matmul partition reduce, num_graphs =
    max(batch)+1 on device (reduce_max over the tail of the sorted batch),
    fused final scalar chain, out [1,1] f32.
  Total ~32.8us HW exec (sum of both launches); rel err ~4e-6 vs the fp32
  reference.  Engine-queue FIFO order is load-bearing: ops are emitted in
  critical-path order per engine.
"""

import numpy as np

import concourse.bacc as bacc
import concourse.mybir as mybir
import concourse.tile as tile
from concourse import bass_utils

F32 = mybir.dt.float32
F16 = mybir.dt.float16
ALU = mybir.AluOpType
ACT = mybir.ActivationFunctionType
AX = mybir.AxisListType

N_NODES = 4000
N_EDGES = 6000
N_CORES = 8
N_PAD = 4096          # 128 * 32
HI = 32               # node hi-digits
LO = 128              # node lo-digits
PENALTY_SCALE = 16 * 200 * 3   # 9600
PAD_NODES = N_PAD - N_NODES    # 96 padded nodes, each contributes exp(0)=1

EPC = N_EDGES // N_CORES       # 750 edges per core
TPC = (EPC + 127) // 128       # 6 edge tiles per core

def _build_phase1(T: int):
    """Per-core partial computation: out 'partial' [128, 65] f16."""
    nc = bacc.Bacc("TRN2", target_bir_lowering=False, debug=False, num_devices=1)

    # blob: [tlo,ulo,thi,uhi (4T) | tf,uf,p (3T)] -- iotas/constants are
    # generated on-device, so the only input DMA is the edge data itself
    blobd = nc.dram_tensor("blob", [128, 7 * T], F32, kind="ExternalInput").ap()
    partiald = nc.dram_tensor("partial", [128, 65], F16, kind="ExternalOutput").ap()

    with tile.TileContext(nc) as tc:
        with (
            tc.tile_pool(name="const", bufs=1) as cpool,
            tc.tile_pool(name="work", bufs=1) as wpool,
            tc.tile_pool(name="psum", bufs=1, space="PSUM") as ppool,
        ):
            # warm the Ln ACT table while the input DMA is in flight
            wz = cpool.tile([128, 1], F32, tag="wz")
            nc.vector.memset(wz[:], 0.5)
            wb = cpool.tile([128, 1], F32, tag="wb")
            nc.gpsimd.memset(wb[:], 0.0)
            bias1 = cpool.tile([128, 1], F32, tag="bias1")
            nc.gpsimd.memset(bias1[:], 1.0 + 1e-6)
            wo = cpool.tile([128, 1], F32, tag="wo")
            nc.scalar.activation(wo[:], wz[:], ACT.Ln, bias=wb[:])
            # iotas generated on-device (no DMA dependency)
            ioi = cpool.tile([128, LO], mybir.dt.int32, tag="ioi")
            nc.gpsimd.iota(ioi[:], pattern=[[1, LO]], base=0, channel_multiplier=0)
            io128t = cpool.tile([128, LO], F32, tag="io128t")
            nc.gpsimd.tensor_copy(io128t[:], ioi[:])
            io128 = io128t[:]
            io32 = io128t[:, 0:HI]

            bb = cpool.tile([128, 7 * T], F32, tag="bb")
            nc.sync.dma_start(bb[:], blobd)
            lo_pair = bb[:, 0:2 * T]
            hi_pair = bb[:, 2 * T:4 * T]
            tf = bb[:, 4 * T:5 * T]
            uf = bb[:, 5 * T:6 * T]
            pp = bb[:, 6 * T:7 * T]

            C = wpool.tile([128, 65], F16, tag="C")

            # ---- one-hots (f16, exact), few wide DVE ops via stride-0 APs
            H_all = wpool.tile([128, 2 * T * HI], F16, tag="H_all")
            nc.vector.tensor_tensor(
                H_all[:].rearrange("p (t h) -> p t h", h=HI),
                io32.rearrange("p (o h) -> p o h", o=1).to_broadcast((128, 2 * T, HI)),
                hi_pair.rearrange("p (t o) -> p t o", o=1).to_broadcast((128, 2 * T, HI)),
                op=ALU.is_equal,
            )
            # lo one-hot, target half first so the MM1 group can start early
            A_all = wpool.tile([128, 2 * T * LO], F16, tag="A_all")
            nc.vector.tensor_tensor(
                A_all[:, 0:T * LO].rearrange("p (t l) -> p t l", l=LO),
                io128.rearrange("p (o l) -> p o l", o=1).to_broadcast((128, T, LO)),
                lo_pair[:, 0:T].rearrange("p (t o) -> p t o", o=1)
                    .to_broadcast((128, T, LO)),
                op=ALU.is_equal,
            )
            # V = [logmsg | p] on the ACT engine (parallel to the DVE ops)
            V = wpool.tile([128, 2 * T], F32, tag="V")
            nc.scalar.activation(V[:, 0:T], pp, ACT.Ln, scale=-1.0, bias=bias1[:])
            nc.scalar.copy(V[:, T:2 * T], pp)

            # RS_all: per tile i the contiguous [rp_i(32) | rst_i(32)]
            RS_all = wpool.tile([128, T * 64], F16, tag="RS_all")
            nc.vector.tensor_tensor(
                RS_all[:].rearrange("p (t o h) -> p o t h", o=2, h=HI),
                H_all[:, 0:T * HI].rearrange("p (o t h) -> p o t h", o=1, h=HI)
                    .to_broadcast((128, 2, T, HI)),
                V[:].rearrange("p (o t) -> p o t", o=2)
                    .rearrange("p o (t h) -> p o t h", h=1)
                    .to_broadcast((128, 2, T, HI)),
                op=ALU.mult,
            )
            # source half of the lo one-hot + small per-edge prep
            nc.vector.tensor_tensor(
                A_all[:, T * LO:2 * T * LO].rearrange("p (t l) -> p t l", l=LO),
                io128.rearrange("p (o l) -> p o l", o=1).to_broadcast((128, T, LO)),
                lo_pair[:, T:2 * T].rearrange("p (t o) -> p t o", o=1)
                    .to_broadcast((128, T, LO)),
                op=ALU.is_equal,
            )
            m = wpool.tile([128, T], F32, tag="m")
            nc.vector.tensor_tensor(m[:], tf, uf, op=ALU.is_equal)
            valu = wpool.tile([128, T], F32, tag="valu")   # p * (1 - m)
            nc.vector.scalar_tensor_tensor(
                valu[:], m[:], 0.5, pp, op0=ALU.is_lt, op1=ALU.mult
            )
            rsu_all = wpool.tile([128, T * HI], F16, tag="rsu_all")
            nc.vector.tensor_tensor(
                rsu_all[:].rearrange("p (t h) -> p t h", h=HI),
                H_all[:, T * HI:2 * T * HI].rearrange("p (t h) -> p t h", h=HI),
                valu[:].rearrange("p (t o) -> p t o", o=1).to_broadcast((128, T, HI)),
                op=ALU.mult,
            )
            # dp2 = p^2 (2 - m) = (valu + p) * p, row-summed (off critical path)
            tsum = wpool.tile([128, T], F32, tag="tsum")
            nc.vector.tensor_tensor(tsum[:], valu[:], pp, op=ALU.add)
            dp2scr = wpool.tile([128, T], F32, tag="dp2scr")
            dp2r = wpool.tile([128, 1], F32, tag="dp2r")
            nc.vector.scalar_tensor_tensor(
                dp2scr[:], tsum[:], 1.0, pp,
                op0=ALU.mult, op1=ALU.mult, accum_out=dp2r[:],
            )

            # ---- scatter-add matmuls: P12 = [log_score(32) | s(32)]
            P12 = ppool.tile([128, 64], F32, tag="P12")
            for i in range(T):
                nc.tensor.matmul(
                    P12[:, 0:64],
                    A_all[:, i * LO:(i + 1) * LO],
                    RS_all[:, i * 64:(i + 1) * 64],
                    start=(i == 0), stop=False, skip_group_check=True,
                )
            for i in range(T):
                nc.tensor.matmul(
                    P12[:, 32:64],
                    A_all[:, (T + i) * LO:(T + i + 1) * LO],
                    rsu_all[:, i * HI:(i + 1) * HI],
                    start=False, stop=(i == T - 1), skip_group_check=True,
                )

            nc.scalar.copy(C[:, 0:64], P12[:])
            nc.gpsimd.tensor_copy(C[:, 64:65], dp2r[:])
            nc.sync.dma_start(partiald, C[:])

    nc.compile()
    return nc


def _build_phase2():
    """Combine 8 partials -> final scalar. Runs on one core."""
    nc = bacc.Bacc("TRN2", target_bir_lowering=False, debug=False, num_devices=1)

    # partials, c innermost: partsa = x 0:32 (log_score), partsb = x 32:65
    # (s | dp2) then 64 cols whose row 0 holds batch[-64:] (batch is sorted
    # by construction, so max(batch) = max of that tail; values < 32 are
    # exact in f16)
    partsad = nc.dram_tensor("partsa", [128, 256], F16, kind="ExternalInput").ap()
    partsbd = nc.dram_tensor("partsb", [128, 328], F16, kind="ExternalInput").ap()
    outd = nc.dram_tensor("out", [1, 1], F32, kind="ExternalOutput").ap()

    with tile.TileContext(nc) as tc:
        with (
            tc.tile_pool(name="pool", bufs=1) as pool,
            tc.tile_pool(name="psum", bufs=1, space="PSUM") as ppool,
        ):
            wz = pool.tile([128, 1], F32, tag="wz")
            nc.vector.memset(wz[:], 0.5)
            wb = pool.tile([128, 1], F32, tag="wb")
            nc.gpsimd.memset(wb[:], 0.0)
            wo = pool.tile([128, 1], F32, tag="wo")
            nc.scalar.activation(wo[:], wz[:], ACT.Exp, bias=wb[:])

            ones_t = pool.tile([128, 1], F32, tag="ones_t")
            nc.gpsimd.memset(ones_t[:], 1.0)
            bzero = wb[:]

            # two input DMAs on independent queues (sync + gpsimd)
            pta = pool.tile([128, 256], F16, tag="pta")
            nc.sync.dma_start(pta[:], partsad)
            ptb = pool.tile([128, 328], F16, tag="ptb")
            nc.gpsimd.dma_start(ptb[:], partsbd)

            # 8-way partial sums; the log_score half unblocks EXP first
            C2a = pool.tile([128, 32], F32, tag="C2a")
            nc.vector.tensor_reduce(
                C2a[:], pta[:].rearrange("p (x c) -> p x c", c=8),
                axis=AX.X, op=ALU.add,
            )
            C2b = pool.tile([128, 33], F32, tag="C2b")
            nc.vector.tensor_reduce(
                C2b[:], ptb[:, 0:264].rearrange("p (x c) -> p x c", c=8),
                axis=AX.X, op=ALU.add,
            )

            R = pool.tile([128, 3], F32, tag="R")
            scr1 = pool.tile([128, HI], F32, tag="scr1")
            nc.scalar.activation(scr1[:], C2a[:], ACT.Exp, bias=bzero,
                                 accum_out=R[:, 0:1])
            scr2 = pool.tile([128, HI], F32, tag="scr2")
            nc.vector.scalar_tensor_tensor(
                scr2[:], C2b[:, 0:32], 1.0, C2b[:, 0:32],
                op0=ALU.mult, op1=ALU.mult, accum_out=R[:, 1:2],
            )
            nc.vector.tensor_copy(R[:, 2:3], C2b[:, 32:33])

            # num_graphs: rng = 100 / (max(batch) + 1), off the critical path
            ng = pool.tile([1, 1], F32, tag="ng")
            nc.vector.tensor_reduce(ng[:], ptb[0:1, 264:328], axis=AX.X, op=ALU.max)
            ng1 = pool.tile([1, 1], F32, tag="ng1")
            nc.vector.tensor_scalar(ng1[:], ng[:], 1.0, 0.01, op0=ALU.add, op1=ALU.mult)
            rng = pool.tile([1, 1], F32, tag="rng")
            nc.vector.reciprocal(rng[:], ng1[:])

            F = ppool.tile([1, 3], F32, tag="F")
            nc.tensor.matmul(F[:], ones_t[:], R[:], start=True, stop=True)
            Fs = pool.tile([1, 2], F32, tag="Fs")
            nc.scalar.copy(Fs[:], F[:, 1:3])

            l2 = pool.tile([1, 1], F32, tag="l2")
            SC = PENALTY_SCALE / N_NODES
            nc.scalar.activation(l2[:], F[:, 0:1], ACT.Copy,
                                 bias=-float(PAD_NODES) * SC, scale=SC)
            d32 = pool.tile([1, 1], F32, tag="d32")
            nc.vector.tensor_tensor(d32[:], Fs[:, 0:1], Fs[:, 1:2], op=ALU.subtract)
            # res = d32 * (100/ng) + l2 in one fused op (scalar is an AP)
            res = pool.tile([1, 1], F32, tag="res")
            nc.vector.scalar_tensor_tensor(
                res[:], d32[:], rng[:], l2[:], op0=ALU.mult, op1=ALU.add
            )
            nc.sync.dma_start(outd, res[:])

    nc.compile()
    return nc


def _pack_core(tt, uu, p, T):
    """Pack one core's edge shard into the [128, 7*T] fp32 edata layout."""
    ne = tt.shape[0]
    npad = T * 128

    def pad(a, fill):
        out = np.full(npad, fill, np.float64)
        out[:ne] = a
        return out.reshape(T, 128).T.astype(np.float32)  # [128, T]

    t_lo = pad(tt % 128, 0.0)
    t_hi = pad(tt // 128, float(HI))     # sentinel hi -> matches nothing
    u_lo = pad(uu % 128, 0.0)
    u_hi = pad(uu // 128, float(HI))
    tf = pad(tt, 0.0)
    uf = pad(uu, 0.0)                    # pad: tf==uf -> m=1, but p=0
    pf = pad(p, 0.0)
    return np.concatenate([t_lo, u_lo, t_hi, u_hi, tf, uf, pf], axis=1)


_CACHE = {}


def _get(name, builder, *a):
    if name not in _CACHE:
        _CACHE[name] = builder(*a)
    return _CACHE[name]


def kernel(x, edge_index, edge_feature, batch, _trace=False):
    x = np.asarray(x)
    ei = np.asarray(edge_index).astype(np.int64)
    p = np.asarray(edge_feature).astype(np.float32)[:, 0]
    batch = np.asarray(batch).astype(np.int64)

    uu_all = ei[0].astype(np.float64)
    tt_all = ei[1].astype(np.float64)

    # ---- phase 1: per-core partials (no cross-core dependencies)
    nc1 = _get("p1", _build_phase1, TPC)
    in_maps = []
    for c in range(N_CORES):
        sl = slice(c * EPC, (c + 1) * EPC)
        in_maps.append({"blob": _pack_core(tt_all[sl], uu_all[sl], p[sl], TPC)})
    r1 = bass_utils.run_bass_kernel_spmd(
        nc1, in_maps, core_ids=list(range(N_CORES)), trace=_trace
    )

    # gather/unshard the per-core partials (pure data movement)
    parts = np.stack(
        [np.asarray(r1.results[c]["partial"]) for c in range(N_CORES)], axis=2
    ).astype(np.float16)                               # [p, x, c], c innermost

    # ---- phase 2: combine on one core
    nc2 = _get("p2", _build_phase2)
    btail = np.zeros((128, 64), np.float16)
    btail[0, :] = batch[-64:].astype(np.float16)
    partsa = parts[:, 0:32, :].reshape(128, 256)
    partsb = np.concatenate([parts[:, 32:65, :].reshape(128, 264), btail], axis=1)
    r2 = bass_utils.run_bass_kernel_spmd(
        nc2, [{"partsa": partsa, "partsb": partsb}], core_ids=[0], trace=_trace,
    )
    out = np.asarray(r2.results[0]["out"], dtype=np.float32).reshape(1, 1)
    if _trace:
        kernel.last_results = (r1, r2)
    return out
